# revision 26
# baseline (speedup 1.0000x reference)
"""Multi-head attention (B=8, N=1024, C=768, H=12) on 8 TRN2 NeuronCores.

Sharding: pure data-parallel over batch — core i computes batch element i
with replicated weights. No collectives.

Per-core kernel (x: [1024, 768]):
  - xT = x.T via DMA-xbar transpose (bf16 DRAM roundtrip), frees the PE
  - qkT[n, m] = (x @ w_qkv[:, :1536]).T   (channels on partitions)
  - v[m, n]   = x @ w_qkv[:, 1536:]       (tokens on partitions), with a
    ones-column appended per head, zero-padded to 128 lhsT columns so the
    U matmuls take the full-square fast path
  - per head pair: ST[j, i] = k_h^T q_h with k zero-padded to [128, 128]
    full-square lhsT tiles (non-square lhsT runs at half rate on TRN2),
    E = exp(ST/8) in bf16 on the ACT engine (the saturated engine of the
    attention phase), U'[*, i] = v'^T E accumulated over j in PSUM
    (row 64 = softmax denominator r via the ones-column).
    UT emission is software-pipelined one j behind ST/exp so the PE never
    waits on the current exp; the next pair's qkT runs at the pair
    boundary with its PSUM->bf16 copies on the then-idle ACT engine.
  - O = U[0:64]/r via approx-reciprocal + gpsimd partition-broadcast +
    DVE multiply, stored as OT pairs [128-channels, tokens] (= proj lhsT)
  - out = OT.T @ w_proj + b_proj

Measured: ~245-265 us HW exec (run-to-run variance +-7%), rel err 5.4e-3
vs the f32 reference (bf16 compute, f32 accumulation).
"""

import functools

import numpy as np

import concourse.bass as bass
import concourse.mybir as mybir
from concourse import bacc
from concourse.tile import TileContext
from concourse.bass_utils import run_bass_kernel_spmd

B, N, C, H = 8, 1024, 768, 12
D = C // H  # 64
SCALE = float(D) ** -0.5
F32 = mybir.dt.float32
BF16 = mybir.dt.bfloat16

KT = C // 128      # 6  contraction tiles over channels
MT = N // 128      # 8  token tiles
PAIRS = H // 2     # 6  head pairs


def _build():
    nc = bacc.Bacc(None, target_bir_lowering=False, debug=False)
    x_ext = nc.declare_dram_parameter("x", [N, C], F32, isOutput=False)
    wqkv_ext = nc.declare_dram_parameter("w_qkv", [C, 3 * C], F32, isOutput=False)
    wproj_ext = nc.declare_dram_parameter("w_proj", [C, C], F32, isOutput=False)
    bias_ext = nc.declare_dram_parameter("b_proj", [C], F32, isOutput=False)
    out_ext = nc.declare_dram_parameter("out", [N, C], F32, isOutput=True)

    with TileContext(nc) as tc:
        with (
            tc.tile_pool(name="singles", bufs=1) as singles,
            tc.tile_pool(name="stage", bufs=5) as stage,
            tc.tile_pool(name="xbf", bufs=2) as xbfp,
            tc.tile_pool(name="xt", bufs=1) as xtp,
            tc.tile_pool(name="qkt", bufs=2) as qktp,
            tc.tile_pool(name="vp", bufs=MT) as vpp,
            tc.tile_pool(name="et", bufs=6) as etp,
            tc.tile_pool(name="u", bufs=2 * PAIRS) as up,
            tc.tile_pool(name="small", bufs=3) as smallp,
            tc.tile_pool(name="outp", bufs=2) as outp,
            tc.tile_pool(name="dram", bufs=1, space="DRAM") as dramp,
            tc.tile_pool(name="ps", bufs=2, space="PSUM") as ps,
        ):
            # ---- x: load, cast to bf16, DMA-xbar transpose via DRAM ----
            xdram = dramp.tile([N, C], BF16)
            for m in range(MT):
                st_x = stage.tile([128, C], F32, tag="stx")
                for ch in range(2):
                    nc.sync.dma_start(
                        out=st_x[:, ch * 384:(ch + 1) * 384],
                        in_=x_ext[m * 128:(m + 1) * 128, ch * 384:(ch + 1) * 384])
                xb = xbfp.tile([128, C], BF16, tag="xbf")
                nc.vector.tensor_copy(out=xb, in_=st_x)
                nc.sync.dma_start(out=xdram[m * 128:(m + 1) * 128, :], in_=xb)
            xt = [xtp.tile([128, N], BF16, tag=f"xt{k}", name=f"xt{k}")
                  for k in range(KT)]
            for k in range(KT):
                nc.sync.dma_start_transpose(
                    xt[k], xdram[:, k * 128:(k + 1) * 128])

            # ---- weights: w_v columns first so v' can start early ----
            wv = []     # 6 x [128, 768]   rhs for v
            for k in range(KT):
                st_v = stage.tile([128, C], F32, tag="stage", name=f"stv{k}")
                for ch in range(2):
                    nc.sync.dma_start(
                        out=st_v[:, ch * 384:(ch + 1) * 384],
                        in_=wqkv_ext[k * 128:(k + 1) * 128,
                                     2 * C + ch * 384:2 * C + (ch + 1) * 384])
                t_v = singles.tile([128, C], BF16, tag=f"wv{k}", name=f"wv{k}")
                nc.scalar.copy(out=t_v, in_=st_v)
                wv.append(t_v)

            def emit_wqk():
                wqk = []
                for k in range(KT):
                    st_qk = stage.tile([128, 2 * C], F32, tag="stage",
                                       name=f"stqk{k}")
                    for ch in range(3):
                        nc.sync.dma_start(
                            out=st_qk[:, ch * 512:(ch + 1) * 512],
                            in_=wqkv_ext[k * 128:(k + 1) * 128,
                                         ch * 512:(ch + 1) * 512])
                    t_qk = singles.tile([128, 2 * C], BF16, tag=f"wqk{k}",
                                        name=f"wqk{k}")
                    nc.scalar.copy(out=t_qk, in_=st_qk)
                    wqk.append(t_qk)
                return wqk

            # ---- v' = [x @ w_v | ones | zero-pad] per head ----
            vp = []
            for m in range(MT):
                pv = ps.tile([128, N], F32, tag="ut" if m % 2 else "ut2", bufs=1)
                for k in range(KT):
                    lhsT = xt[k][:, m * 128:(m + 1) * 128]
                    nc.tensor.matmul(pv[:, 0:512], lhsT, wv[k][:, 0:512],
                                     start=(k == 0), stop=(k == KT - 1))
                    nc.tensor.matmul(pv[:, 512:768], lhsT, wv[k][:, 512:768],
                                     start=(k == 0), stop=(k == KT - 1))
                t_vp = vpp.tile([128, H, 128], BF16, tag="vp")
                nc.vector.tensor_copy(
                    out=t_vp[:, :, 0:D],
                    in_=pv[:, 0:C].rearrange("p (h d) -> p h d", h=H))
                nc.vector.memset(t_vp[:, :, D:D + 1], 1.0)
                nc.vector.memset(t_vp[:, :, D + 1:128], 0.0)
                vp.append(t_vp)

            wqk = emit_wqk()

            # ---- w_proj / bias (loaded lazily during pair 1) ----
            wpr = []

            def emit_wproj():
                for k in range(KT):
                    st_pr = stage.tile([128, C], F32, tag="stage", name=f"stpr{k}")
                    for ch in range(2):
                        nc.sync.dma_start(
                            out=st_pr[:, ch * 384:(ch + 1) * 384],
                            in_=wproj_ext[k * 128:(k + 1) * 128,
                                          ch * 384:(ch + 1) * 384])
                    t_pr = singles.tile([128, C], BF16, tag=f"wpr{k}",
                                        name=f"wpr{k}")
                    nc.vector.tensor_copy(out=t_pr, in_=st_pr)
                    wpr.append(t_pr)
                bias_tile = singles.tile([128, C], F32, name="bias_bc")
                nc.sync.dma_start(out=bias_tile,
                                  in_=bias_ext[:].partition_broadcast(128))
                return bias_tile

            # ---- per head pair: qkT then attention ----
            upairs = {}  # (pair, ihalf) -> [128, 512] bf16 OT tile

            def _qk_psum(t):
                pq = ps.tile([128, N], F32, tag="st", name=f"pq{t}")
                for k in range(KT):
                    lhsT = wqk[k][:, t * 128:(t + 1) * 128]
                    nc.tensor.matmul(pq[:, 0:512], lhsT, xt[k][:, 0:512],
                                     start=(k == 0), stop=(k == KT - 1))
                    nc.tensor.matmul(pq[:, 512:1024], lhsT, xt[k][:, 512:1024],
                                     start=(k == 0), stop=(k == KT - 1))
                return pq

            def emit_qkt_q(p):
                pq = _qk_psum(p)
                t_qk = qktp.tile([128, N], BF16, tag="qt", name=f"qt{p}")
                nc.scalar.copy(out=t_qk, in_=pq)
                return t_qk

            def emit_qkt_k(p):
                pq = _qk_psum(PAIRS + p)
                ka_t = qktp.tile([128, N], BF16, tag="ka", name=f"ka{p}")
                nc.scalar.copy(out=ka_t[0:64, :], in_=pq[0:64, :])
                if p < 2:
                    nc.vector.memset(ka_t[64:128, :], 0.0)
                kb_t = qktp.tile([128, N], BF16, tag="kb", name=f"kb{p}")
                if p < 2:
                    nc.vector.memset(kb_t[0:64, :], 0.0)
                nc.scalar.copy(out=kb_t[64:128, :], in_=pq[64:128, :])
                return ka_t, kb_t

            pending_q = emit_qkt_q(0)
            pending_k = emit_qkt_k(0)
            bias_bc = None
            for p in range(PAIRS):
                qtile = pending_q
                ktile_a, ktile_b = pending_k

                # U' accumulators for both heads (full i-range, 2 banks each)
                ut_a = ps.tile([128, N], F32, tag="ut", bufs=1, name=f"uta{p}")
                ut_b = ps.tile([128, N], F32, tag="ut2", bufs=1, name=f"utb{p}")

                ets = []  # (et_a, et_b) per j

                def emit_ut(j, ets=ets, ut_a=ut_a, ut_b=ut_b, p=p):
                    et_a, et_b = ets[j]
                    for (ut, et, h) in ((ut_a, et_a, 2 * p), (ut_b, et_b, 2 * p + 1)):
                        for ih in range(2):
                            sl = slice(ih * 512, (ih + 1) * 512)
                            nc.tensor.matmul(ut[:, sl], vp[j][:, h, :],
                                             et[:, sl],
                                             start=(j == 0), stop=(j == MT - 1))

                for j in range(MT):
                    st_a = ps.tile([128, N], F32, tag="st", name=f"sta{p}_{j}")
                    st_b = ps.tile([128, N], F32, tag="st", name=f"stb{p}_{j}")
                    ka = ktile_a[:, j * 128:(j + 1) * 128]
                    kb = ktile_b[:, j * 128:(j + 1) * 128]
                    for st_t, kk in ((st_a, ka), (st_b, kb)):
                        for ih in range(2):
                            sl = slice(ih * 512, (ih + 1) * 512)
                            nc.tensor.matmul(st_t[:, sl], kk, qtile[:, sl],
                                             start=True, stop=True)
                    et_a = etp.tile([128, N], BF16, tag="et", name=f"eta{p}_{j}")
                    et_b = etp.tile([128, N], BF16, tag="et", name=f"etb{p}_{j}")
                    nc.scalar.activation(
                        out=et_a, in_=st_a,
                        func=mybir.ActivationFunctionType.Exp, scale=SCALE)
                    nc.scalar.activation(
                        out=et_b, in_=st_b,
                        func=mybir.ActivationFunctionType.Exp, scale=SCALE)
                    ets.append((et_a, et_b))
                    # software-pipeline: consume last j's E while this j's exp runs
                    if j > 0:
                        emit_ut(j - 1)
                    if j == 2 and p == 1:
                        bias_bc = emit_wproj()
                # prefetch next pair's q/k; q first (its psum slot frees
                # after exp_a(7), before UT(7)'s exp_b dependency)
                if p + 1 < PAIRS:
                    pending_q = emit_qkt_q(p + 1)
                emit_ut(MT - 1)
                if p + 1 < PAIRS:
                    pending_k = emit_qkt_k(p + 1)

                # normalize: O = U[0:64] / r, packed [128, 512] per i-half
                for ih in range(2):
                    sl = slice(ih * 512, (ih + 1) * 512)
                    t_u = up.tile([128, 512], BF16, tag="u", name=f"u{p}_{ih}")
                    for hh, ut in ((0, ut_a), (1, ut_b)):
                        r_sb = smallp.tile([1, 512], F32, tag="rsb")
                        nc.vector.tensor_copy(out=r_sb, in_=ut[D:D + 1, sl])
                        rinv = smallp.tile([1, 512], F32, tag="rinv")
                        nc.vector.reciprocal_approx_fast(out=rinv, in_=r_sb)
                        rb = smallp.tile([64, 512], F32, tag="rb")
                        nc.gpsimd.partition_broadcast(rb, rinv)
                        nc.vector.tensor_mul(
                            out=t_u[hh * 64:(hh + 1) * 64, :],
                            in0=ut[0:D, sl], in1=rb)
                    upairs[(p, ih)] = t_u

            # ---- proj + bias ----
            for m in range(MT):
                pp = ps.tile([128, N], F32, tag="st")
                ih, off = m // 4, (m % 4) * 128
                for p in range(PAIRS):
                    lhsT = upairs[(p, ih)][:, off:off + 128]
                    nc.tensor.matmul(pp[:, 0:512], lhsT, wpr[p][:, 0:512],
                                     start=(p == 0), stop=(p == PAIRS - 1))
                    nc.tensor.matmul(pp[:, 512:768], lhsT, wpr[p][:, 512:768],
                                     start=(p == 0), stop=(p == PAIRS - 1))
                t_o = outp.tile([128, C], F32, tag="out")
                nc.vector.tensor_add(out=t_o, in0=pp[:, 0:C], in1=bias_bc)
                for ch in range(2):
                    nc.sync.dma_start(
                        out=out_ext[m * 128:(m + 1) * 128,
                                    ch * 384:(ch + 1) * 384],
                        in_=t_o[:, ch * 384:(ch + 1) * 384])

    nc.compile()
    return nc


@functools.cache
def _built():
    return _build()


def _run(inputs, trace=False, trace_cores=None):
    nc = _built()
    x = np.ascontiguousarray(np.asarray(inputs["x"], dtype=np.float32))
    w_qkv = np.ascontiguousarray(np.asarray(inputs["w_qkv"], dtype=np.float32))
    w_proj = np.ascontiguousarray(np.asarray(inputs["w_proj"], dtype=np.float32))
    b_proj = np.ascontiguousarray(np.asarray(inputs["b_proj"], dtype=np.float32))
    in_maps = [
        {"x": x[i], "w_qkv": w_qkv, "w_proj": w_proj, "b_proj": b_proj}
        for i in range(B)
    ]
    res = run_bass_kernel_spmd(
        nc, in_maps, core_ids=list(range(B)), trace=trace,
        trace_cores=trace_cores,
    )
    out = np.stack([res.results[i]["out"] for i in range(B)], axis=0)
    return out, res


def kernel(**inputs) -> np.ndarray:
    out, _ = _run(inputs, trace=False)
    return out


# revision 28
# speedup vs baseline: 1.0109x; 1.0109x over previous
"""Multi-head attention (B=8, N=1024, C=768, H=12) on 8 TRN2 NeuronCores.

Sharding: pure data-parallel over batch — core i computes batch element i
with replicated weights. No collectives.

Per-core kernel (x: [1024, 768]):
  - xT = x.T via DMA-xbar transpose (bf16 DRAM roundtrip), frees the PE
  - qkT[n, m] = (x @ w_qkv[:, :1536]).T   (channels on partitions)
  - v[m, n]   = x @ w_qkv[:, 1536:]       (tokens on partitions), with a
    ones-column appended per head, zero-padded to 128 lhsT columns so the
    U matmuls take the full-square fast path
  - per head pair: ST[j, i] = k_h^T q_h with k zero-padded to [128, 128]
    full-square lhsT tiles (non-square lhsT runs at half rate on TRN2),
    E = exp(ST/8) in bf16 on the ACT engine (the saturated engine of the
    attention phase), U'[*, i] = v'^T E accumulated over j in PSUM
    (row 64 = softmax denominator r via the ones-column).
    UT emission is software-pipelined one j behind ST/exp so the PE never
    waits on the current exp; the next pair's qkT runs at the pair
    boundary with its PSUM->bf16 copies on the then-idle ACT engine.
  - O = U[0:64]/r via approx-reciprocal + gpsimd partition-broadcast +
    DVE multiply, stored as OT pairs [128-channels, tokens] (= proj lhsT)
  - out = OT.T @ w_proj + b_proj

Measured: ~245-265 us HW exec (run-to-run variance +-7%), rel err 5.4e-3
vs the f32 reference (bf16 compute, f32 accumulation).
"""

import functools

import numpy as np

import concourse.bass as bass
import concourse.mybir as mybir
from concourse import bacc
from concourse.tile import TileContext
from concourse.bass_utils import run_bass_kernel_spmd

B, N, C, H = 8, 1024, 768, 12
D = C // H  # 64
SCALE = float(D) ** -0.5
F32 = mybir.dt.float32
BF16 = mybir.dt.bfloat16

KT = C // 128      # 6  contraction tiles over channels
MT = N // 128      # 8  token tiles
PAIRS = H // 2     # 6  head pairs


def _build():
    nc = bacc.Bacc(None, target_bir_lowering=False, debug=False)
    x_ext = nc.declare_dram_parameter("x", [N, C], F32, isOutput=False)
    wqkv_ext = nc.declare_dram_parameter("w_qkv", [C, 3 * C], F32, isOutput=False)
    wproj_ext = nc.declare_dram_parameter("w_proj", [C, C], F32, isOutput=False)
    bias_ext = nc.declare_dram_parameter("b_proj", [C], F32, isOutput=False)
    out_ext = nc.declare_dram_parameter("out", [N, C], F32, isOutput=True)

    with TileContext(nc) as tc:
        with (
            tc.tile_pool(name="singles", bufs=1) as singles,
            tc.tile_pool(name="stage", bufs=5) as stage,
            tc.tile_pool(name="xbf", bufs=2) as xbfp,
            tc.tile_pool(name="xt", bufs=1) as xtp,
            tc.tile_pool(name="qkt", bufs=2) as qktp,
            tc.tile_pool(name="vp", bufs=MT) as vpp,
            tc.tile_pool(name="et", bufs=4) as etp,
            tc.tile_pool(name="u", bufs=2 * PAIRS) as up,
            tc.tile_pool(name="small", bufs=3) as smallp,
            tc.tile_pool(name="outp", bufs=2) as outp,
            tc.tile_pool(name="dram", bufs=1, space="DRAM") as dramp,
            tc.tile_pool(name="ps", bufs=2, space="PSUM") as ps,
        ):
            # ---- x: load, cast to bf16, DMA-xbar transpose via DRAM ----
            xdram = dramp.tile([N, C], BF16)
            for m in range(MT):
                st_x = stage.tile([128, C], F32, tag="stx")
                for ch in range(2):
                    nc.sync.dma_start(
                        out=st_x[:, ch * 384:(ch + 1) * 384],
                        in_=x_ext[m * 128:(m + 1) * 128, ch * 384:(ch + 1) * 384])
                xb = xbfp.tile([128, C], BF16, tag="xbf")
                nc.vector.tensor_copy(out=xb, in_=st_x)
                nc.sync.dma_start(out=xdram[m * 128:(m + 1) * 128, :], in_=xb)
            xt = [xtp.tile([128, N], BF16, tag=f"xt{k}", name=f"xt{k}")
                  for k in range(KT)]
            for k in range(KT):
                nc.sync.dma_start_transpose(
                    xt[k], xdram[:, k * 128:(k + 1) * 128])

            # ---- weights: w_v columns first so v' can start early ----
            wv = []     # 6 x [128, 768]   rhs for v
            for k in range(KT):
                st_v = stage.tile([128, C], F32, tag="stage", name=f"stv{k}")
                for ch in range(2):
                    nc.sync.dma_start(
                        out=st_v[:, ch * 384:(ch + 1) * 384],
                        in_=wqkv_ext[k * 128:(k + 1) * 128,
                                     2 * C + ch * 384:2 * C + (ch + 1) * 384])
                t_v = singles.tile([128, C], BF16, tag=f"wv{k}", name=f"wv{k}")
                nc.scalar.copy(out=t_v, in_=st_v)
                wv.append(t_v)

            def emit_wqk():
                wqk = []
                for k in range(KT):
                    st_qk = stage.tile([128, 2 * C], F32, tag="stage",
                                       name=f"stqk{k}")
                    for ch in range(3):
                        nc.sync.dma_start(
                            out=st_qk[:, ch * 512:(ch + 1) * 512],
                            in_=wqkv_ext[k * 128:(k + 1) * 128,
                                         ch * 512:(ch + 1) * 512])
                    t_qk = singles.tile([128, 2 * C], BF16, tag=f"wqk{k}",
                                        name=f"wqk{k}")
                    nc.scalar.copy(out=t_qk, in_=st_qk)
                    wqk.append(t_qk)
                return wqk

            # ---- v' = [x @ w_v | ones | zero-pad] per head ----
            vp = []
            for m in range(MT):
                pv = ps.tile([128, N], F32, tag="ut" if m % 2 else "ut2", bufs=1)
                for k in range(KT):
                    lhsT = xt[k][:, m * 128:(m + 1) * 128]
                    nc.tensor.matmul(pv[:, 0:512], lhsT, wv[k][:, 0:512],
                                     start=(k == 0), stop=(k == KT - 1))
                    nc.tensor.matmul(pv[:, 512:768], lhsT, wv[k][:, 512:768],
                                     start=(k == 0), stop=(k == KT - 1))
                t_vp = vpp.tile([128, H, 128], BF16, tag="vp")
                nc.vector.tensor_copy(
                    out=t_vp[:, :, 0:D],
                    in_=pv[:, 0:C].rearrange("p (h d) -> p h d", h=H))
                nc.vector.memset(t_vp[:, :, D:D + 1], 1.0)
                nc.vector.memset(t_vp[:, :, D + 1:128], 0.0)
                vp.append(t_vp)

            wqk = emit_wqk()

            # ---- w_proj / bias (loaded lazily during pair 1) ----
            wpr = []

            def emit_wproj():
                for k in range(KT):
                    st_pr = stage.tile([128, C], F32, tag="stage", name=f"stpr{k}")
                    for ch in range(2):
                        nc.sync.dma_start(
                            out=st_pr[:, ch * 384:(ch + 1) * 384],
                            in_=wproj_ext[k * 128:(k + 1) * 128,
                                          ch * 384:(ch + 1) * 384])
                    t_pr = singles.tile([128, C], BF16, tag=f"wpr{k}",
                                        name=f"wpr{k}")
                    nc.vector.tensor_copy(out=t_pr, in_=st_pr)
                    wpr.append(t_pr)
                bias_tile = singles.tile([128, C], F32, name="bias_bc")
                nc.sync.dma_start(out=bias_tile,
                                  in_=bias_ext[:].partition_broadcast(128))
                return bias_tile

            # ---- per head pair: qkT then attention ----
            upairs = {}  # (pair, ihalf) -> [128, 512] bf16 OT tile

            def _qk_psum(t):
                pq = ps.tile([128, N], F32, tag="st", name=f"pq{t}")
                for k in range(KT):
                    lhsT = wqk[k][:, t * 128:(t + 1) * 128]
                    nc.tensor.matmul(pq[:, 0:512], lhsT, xt[k][:, 0:512],
                                     start=(k == 0), stop=(k == KT - 1))
                    nc.tensor.matmul(pq[:, 512:1024], lhsT, xt[k][:, 512:1024],
                                     start=(k == 0), stop=(k == KT - 1))
                return pq

            def emit_qkt_q(p):
                pq = _qk_psum(p)
                t_qk = qktp.tile([128, N], BF16, tag="qt", name=f"qt{p}")
                nc.scalar.copy(out=t_qk, in_=pq)
                return t_qk

            def emit_qkt_k(p):
                pq = _qk_psum(PAIRS + p)
                ka_t = qktp.tile([128, N], BF16, tag="ka", name=f"ka{p}")
                nc.scalar.copy(out=ka_t[0:64, :], in_=pq[0:64, :])
                if p < 2:
                    nc.vector.memset(ka_t[64:128, :], 0.0)
                kb_t = qktp.tile([128, N], BF16, tag="kb", name=f"kb{p}")
                if p < 2:
                    nc.vector.memset(kb_t[0:64, :], 0.0)
                nc.scalar.copy(out=kb_t[64:128, :], in_=pq[64:128, :])
                return ka_t, kb_t

            pending_q = emit_qkt_q(0)
            pending_k = emit_qkt_k(0)
            bias_bc = None
            for p in range(PAIRS):
                qtile = pending_q
                ktile_a, ktile_b = pending_k

                # U' accumulators for both heads (full i-range, 2 banks each)
                ut_a = ps.tile([128, N], F32, tag="ut", bufs=1, name=f"uta{p}")
                ut_b = ps.tile([128, N], F32, tag="ut2", bufs=1, name=f"utb{p}")

                ets = []  # (et_a, et_b) per j

                def emit_ut(j, ets=ets, ut_a=ut_a, ut_b=ut_b, p=p):
                    et_a, et_b = ets[j]
                    for (ut, et, h) in ((ut_a, et_a, 2 * p), (ut_b, et_b, 2 * p + 1)):
                        for ih in range(2):
                            sl = slice(ih * 512, (ih + 1) * 512)
                            nc.tensor.matmul(ut[:, sl], vp[j][:, h, :],
                                             et[:, sl],
                                             start=(j == 0), stop=(j == MT - 1))

                for j in range(MT):
                    st_a = ps.tile([128, N], F32, tag="st", name=f"sta{p}_{j}")
                    st_b = ps.tile([128, N], F32, tag="st", name=f"stb{p}_{j}")
                    ka = ktile_a[:, j * 128:(j + 1) * 128]
                    kb = ktile_b[:, j * 128:(j + 1) * 128]
                    for st_t, kk in ((st_a, ka), (st_b, kb)):
                        for ih in range(2):
                            sl = slice(ih * 512, (ih + 1) * 512)
                            nc.tensor.matmul(st_t[:, sl], kk, qtile[:, sl],
                                             start=True, stop=True)
                    et_a = etp.tile([128, N], BF16, tag="et", name=f"eta{p}_{j}")
                    et_b = etp.tile([128, N], BF16, tag="et", name=f"etb{p}_{j}")
                    nc.scalar.activation(
                        out=et_a, in_=st_a,
                        func=mybir.ActivationFunctionType.Exp, scale=SCALE)
                    nc.scalar.activation(
                        out=et_b, in_=st_b,
                        func=mybir.ActivationFunctionType.Exp, scale=SCALE)
                    ets.append((et_a, et_b))
                    # software-pipeline: consume last j's E while this j's exp runs
                    if j > 0:
                        emit_ut(j - 1)
                    if j == 2 and p == 1:
                        bias_bc = emit_wproj()
                # next pair's q-tile first: its psum slot frees after
                # exp_a(7), one exp earlier than UT(7)'s exp_b dependency
                if p + 1 < PAIRS:
                    pending_q = emit_qkt_q(p + 1)
                emit_ut(MT - 1)
                if p + 1 < PAIRS:
                    pending_k = emit_qkt_k(p + 1)

                # normalize: O = U[0:64] / r, packed [128, 512] per i-half
                for ih in range(2):
                    sl = slice(ih * 512, (ih + 1) * 512)
                    t_u = up.tile([128, 512], BF16, tag="u", name=f"u{p}_{ih}")
                    for hh, ut in ((0, ut_a), (1, ut_b)):
                        r_sb = smallp.tile([1, 512], F32, tag="rsb")
                        nc.vector.tensor_copy(out=r_sb, in_=ut[D:D + 1, sl])
                        rinv = smallp.tile([1, 512], F32, tag="rinv")
                        nc.vector.reciprocal_approx_fast(out=rinv, in_=r_sb)
                        rb = smallp.tile([64, 512], F32, tag="rb")
                        nc.gpsimd.partition_broadcast(rb, rinv)
                        nc.vector.tensor_mul(
                            out=t_u[hh * 64:(hh + 1) * 64, :],
                            in0=ut[0:D, sl], in1=rb)
                    upairs[(p, ih)] = t_u

            # ---- proj + bias ----
            for m in range(MT):
                pp = ps.tile([128, N], F32, tag="st")
                ih, off = m // 4, (m % 4) * 128
                for p in range(PAIRS):
                    lhsT = upairs[(p, ih)][:, off:off + 128]
                    nc.tensor.matmul(pp[:, 0:512], lhsT, wpr[p][:, 0:512],
                                     start=(p == 0), stop=(p == PAIRS - 1))
                    nc.tensor.matmul(pp[:, 512:768], lhsT, wpr[p][:, 512:768],
                                     start=(p == 0), stop=(p == PAIRS - 1))
                t_o = outp.tile([128, C], F32, tag="out")
                nc.vector.tensor_add(out=t_o, in0=pp[:, 0:C], in1=bias_bc)
                nc.sync.dma_start(out=out_ext[m * 128:(m + 1) * 128, :], in_=t_o)

    nc.compile()
    return nc


@functools.cache
def _built():
    return _build()


def _run(inputs, trace=False, trace_cores=None):
    nc = _built()
    x = np.ascontiguousarray(np.asarray(inputs["x"], dtype=np.float32))
    w_qkv = np.ascontiguousarray(np.asarray(inputs["w_qkv"], dtype=np.float32))
    w_proj = np.ascontiguousarray(np.asarray(inputs["w_proj"], dtype=np.float32))
    b_proj = np.ascontiguousarray(np.asarray(inputs["b_proj"], dtype=np.float32))
    in_maps = [
        {"x": x[i], "w_qkv": w_qkv, "w_proj": w_proj, "b_proj": b_proj}
        for i in range(B)
    ]
    res = run_bass_kernel_spmd(
        nc, in_maps, core_ids=list(range(B)), trace=trace,
        trace_cores=trace_cores,
    )
    out = np.stack([res.results[i]["out"] for i in range(B)], axis=0)
    return out, res


def kernel(**inputs) -> np.ndarray:
    out, _ = _run(inputs, trace=False)
    return out
